# revision 8
# baseline (speedup 1.0000x reference)
"""DeepHit survival loss on 8 Trainium2 NeuronCores (Bass/Tile).

Math: the O(n^2) pairwise rank loss factorizes. With
  cdf[j,t]  = cumsum_t(exp(phi_j)) / sum(exp(phi_j))          (pad col folded in)
  E[j,t]    = exp(2*cdf[j,t])                                 (sigma = 0.5)
  W[j,d]    = 1{dur_j > d} + 1{dur_j == d}*(1 - ev_j) = 1{d <= dur_j - ev_j}
the pairwise sum  sum_ij rank_mat[i,j]*exp(-r_ij/sigma)  equals
  sum_i ev_i * exp(-2*cdf[i,lab_i]) * D[lab_i, dur_i],   D = E^T @ W  ([256,256]).

Sharding: batch rows n=8192 split as 1024 rows per core. Each core computes a
partial D (256x256) plus per-sample row sums / label-gathers; the host sums the
8 partial Ds, builds the tiny u-weighted histogram P over (lab, dur), takes
<D, P>, and finishes the O(n) nll arithmetic. No collectives needed.
"""

import os
import numpy as np

import concourse.bacc as bacc
import concourse.mybir as mybir
import concourse.tile as tile
from concourse import bass_utils

N, T = 8192, 256
N_CORES = 8
NLOC = N // N_CORES          # 1024 rows per core
NT = NLOC // 128             # 8 partition-tiles per core
ALPHA, SIGMA, EPS = 0.5, 0.5, 1e-7

f32 = mybir.dt.float32
f32r = mybir.dt.float32r
Alu = mybir.AluOpType
Act = mybir.ActivationFunctionType

# float32r matmul streams at full PE rate for N>=256; its operand rounding
# contributes ~4e-6 relative error to D (measured offline). Set False for
# plain (4x slower, exact) fp32 matmuls.
USE_F32R = True

_CACHE = {}
LAST_RESULTS = None


MM_DTYPE = f32r if USE_F32R else f32


def _build():
    nc = bacc.Bacc("TRN2", target_bir_lowering=False, debug=False)

    haz_d = nc.dram_tensor("haz", [NLOC, T], f32, kind="ExternalInput")
    # packed per-tile columns: [:, 0:8] = dur - ev, [:, 8:16] = label
    dpk_d = nc.dram_tensor("dpk", [128, 2 * NT], f32, kind="ExternalInput")
    iota_d = nc.dram_tensor("iota", [128, T], f32, kind="ExternalInput")

    D_d = nc.dram_tensor("D", [T, T], f32, kind="ExternalOutput")
    # [:, 0:8] = cumsum(exp(phi)) at label, [:, 8:16] = rowsum(exp(phi))
    pv_d = nc.dram_tensor("pv", [128, 2 * NT], f32, kind="ExternalOutput")

    with tile.TileContext(nc) as tc:
        with (
            tc.tile_pool(name="const", bufs=1) as cpool,
            tc.tile_pool(name="work", bufs=3) as pool,
            tc.tile_pool(name="stage", bufs=1) as spool,
            tc.tile_pool(name="ps", bufs=1, space="PSUM") as pspool,
        ):
            iota_t = cpool.tile([128, T], f32)
            nc.sync.dma_start(iota_t[:], iota_d[:])
            dpk_t = cpool.tile([128, 2 * NT], f32)
            nc.sync.dma_start(dpk_t[:], dpk_d[:])

            pv_t = spool.tile([128, 2 * NT], f32)
            D0_ps = pspool.tile([128, T], f32)
            D1_ps = pspool.tile([128, T], f32)

            for q in range(NT):
                dme_c = dpk_t[:, q : q + 1]              # dur - ev
                lab_c = dpk_t[:, NT + q : NT + q + 1]
                sume_c = pv_t[:, NT + q : NT + q + 1]    # rowsum(exp) out slot

                haz_t = pool.tile([128, T], f32, tag="haz")
                nc.sync.dma_start(haz_t[:], haz_d[q * 128 : (q + 1) * 128, :])

                # exp(phi) and its row sum (phi max ~5, no overflow; the
                # reference's gamma shift cancels in every ratio used)
                exp_t = pool.tile([128, T], f32, tag="exp")
                nc.scalar.activation(exp_t[:], haz_t[:], Act.Exp, accum_out=sume_c)

                # sequential prefix sum along t (same order as jnp.cumsum)
                cs_t = pool.tile([128, T], f32, tag="cs")
                nc.vector.tensor_tensor_scan(
                    cs_t[:], exp_t[:], exp_t[:], 0.0, Alu.add, Alu.bypass
                )

                # 2 / sum_ng  via  1 / (0.5*sume + 0.5)
                h_c = pool.tile([128, 1], f32, tag="h")
                nc.vector.tensor_scalar(h_c[:], sume_c, 0.5, 0.5, Alu.mult, Alu.add)
                rec2_c = pool.tile([128, 1], f32, tag="rec2")
                nc.vector.reciprocal(rec2_c[:], h_c[:])

                # E = exp(2 * cs / sum_ng)
                E_t = pool.tile([128, T], MM_DTYPE, tag="E")
                nc.scalar.activation(E_t[:], cs_t[:], Act.Exp, scale=rec2_c[:])

                # cum_at = cs[lab] via (iota==lab)*cs then free-axis sum
                scr_t = pool.tile([128, T], f32, tag="scr")
                nc.vector.scalar_tensor_tensor(
                    scr_t[:],
                    iota_t[:],
                    lab_c,
                    cs_t[:],
                    Alu.is_equal,
                    Alu.mult,
                    accum_out=pv_t[:, q : q + 1],
                )

                # W = 1{iota <= dur - ev}
                W_t = pool.tile([128, T], MM_DTYPE, tag="W")
                nc.vector.tensor_scalar(
                    W_t[:], iota_t[:], dme_c, None, Alu.is_le
                )

                # D += E^T @ W, t-chunked over PSUM partitions
                nc.tensor.matmul(
                    D0_ps[:], E_t[:, 0:128], W_t[:],
                    start=(q == 0), stop=(q == NT - 1),
                )
                nc.tensor.matmul(
                    D1_ps[:], E_t[:, 128:T], W_t[:],
                    start=(q == 0), stop=(q == NT - 1),
                )

            D0_sb = spool.tile([128, T], f32)
            D1_sb = spool.tile([128, T], f32)
            nc.scalar.copy(D0_sb[:], D0_ps[:])
            nc.scalar.copy(D1_sb[:], D1_ps[:])
            nc.sync.dma_start(D_d[0:128, :], D0_sb[:])
            nc.sync.dma_start(D_d[128:T, :], D1_sb[:])
            nc.sync.dma_start(pv_d[:], pv_t[:])

    nc.compile()
    return nc


def _get_nc():
    if "nc" not in _CACHE:
        _CACHE["nc"] = _build()
    return _CACHE["nc"]


def _make_in_maps(hazards, duration, event, label):
    iota = np.broadcast_to(
        np.arange(T, dtype=np.float32)[None, :], (128, T)
    ).copy()
    dmef = (duration - event).astype(np.float32)
    labf = label.astype(np.float32)
    in_maps = []
    for c in range(N_CORES):
        sl = slice(c * NLOC, (c + 1) * NLOC)
        dpk = np.empty((128, 2 * NT), np.float32)
        # column q holds rows [c*NLOC + q*128 : c*NLOC + (q+1)*128)
        dpk[:, 0:NT] = dmef[sl].reshape(NT, 128).T
        dpk[:, NT : 2 * NT] = labf[sl].reshape(NT, 128).T
        in_maps.append(
            {
                "haz": np.ascontiguousarray(hazards[sl]),
                "dpk": dpk,
                "iota": iota,
            }
        )
    return in_maps


def _finish_host(hazards, duration, event, label, D_parts, pv_parts):
    """Host glue: O(n) + O(T^2) arithmetic from the per-core device outputs."""
    n = hazards.shape[0]
    dur = duration.astype(np.int64)
    ev = event.astype(np.int64)
    lab = label.astype(np.int64)

    D = np.zeros((T, T), np.float64)
    cum_at_ng = np.empty(n, np.float32)
    sum_ng = np.empty(n, np.float32)
    for c in range(N_CORES):
        D += D_parts[c].astype(np.float64)
        pv = pv_parts[c]  # [128, 16]
        sl = slice(c * NLOC, (c + 1) * NLOC)
        cum_at_ng[sl] = pv[:, 0:NT].T.reshape(NLOC)
        sum_ng[sl] = pv[:, NT : 2 * NT].T.reshape(NLOC) + np.float32(1.0)

    # rank loss: <D, P> with P the u-weighted (lab, dur) histogram
    cdf_at = cum_at_ng.astype(np.float64) / sum_ng.astype(np.float64)
    u = ev * np.exp(-2.0 * cdf_at)
    P = np.zeros((T, T), np.float64)
    np.add.at(P, (lab, dur), u)
    rank_loss = (D * P).sum() / (float(n) * float(n))

    # nll, following the reference formulas exactly
    gamma = np.maximum(hazards.max(axis=1), 0.0).astype(np.float64)
    eg = np.exp(-gamma)
    sum_ = sum_ng * eg
    cum_at = cum_at_ng * eg
    phi_at = hazards[np.arange(n), lab].astype(np.float64)
    evf = ev.astype(np.float64)
    part1 = (phi_at - gamma) * evf
    part2 = -np.log(np.maximum(sum_, 0.0) + EPS)
    part3 = np.log(np.maximum(sum_ - cum_at, 0.0) + EPS) * (1.0 - evf)
    nll = np.mean(-(part1 + part2 + part3))

    return np.float32(ALPHA * nll + (1.0 - ALPHA) * rank_loss)


def kernel(hazards, duration, event, label):
    global LAST_RESULTS
    hazards = np.asarray(hazards, dtype=np.float32)
    duration = np.asarray(duration)
    event = np.asarray(event)
    label = np.asarray(label)

    nc = _get_nc()
    in_maps = _make_in_maps(hazards, duration, event, label)
    trace = bool(int(os.environ.get("KERNEL_TRACE", "0")))
    res = bass_utils.run_bass_kernel_spmd(
        nc,
        in_maps,
        core_ids=list(range(N_CORES)),
        trace=trace,
        trace_cores=list(range(N_CORES)) if trace else None,
        stitch_traces=False,
    )
    LAST_RESULTS = res
    D_parts = [r["D"] for r in res.results]
    pv_parts = [r["pv"] for r in res.results]
    return _finish_host(hazards, duration, event, label, D_parts, pv_parts)


# revision 28
# speedup vs baseline: 1.0390x; 1.0390x over previous
"""DeepHit survival loss on 8 Trainium2 NeuronCores (Bass/Tile).

Math: the O(n^2) pairwise rank loss factorizes. With
  cdf[j,t]  = cumsum_t(exp(phi_j)) / sum(exp(phi_j))          (pad col folded in)
  E[j,t]    = exp(2*cdf[j,t])                                 (sigma = 0.5)
  W[j,d]    = 1{dur_j > d} + 1{dur_j == d}*(1 - ev_j) = 1{d <= dur_j - ev_j}
the pairwise sum  sum_ij rank_mat[i,j]*exp(-r_ij/sigma)  equals
  sum_i ev_i * exp(-2*cdf[i,lab_i]) * D[lab_i, dur_i],   D = E^T @ W  ([256,256]).

Sharding: batch rows n=8192 split as 1024 rows per core. Each core computes a
partial D (256x256) plus per-sample row sums / label-gathers; the host sums the
8 partial Ds, builds the tiny u-weighted histogram P over (lab, dur), takes
<D, P>, and finishes the O(n) nll arithmetic. No collectives needed.

Device structure (per core; 8 row-tiles of 128 rows):
- hazard rows are host-padded to 258 cols with zeros. After the batched exp,
  col 256 is exp(0)=1 (the reference's pad column) and col 257 is a spare.
- per-tile prefix-sum scan whose op1 multiplies by a constant mask (1.0 in
  the body, 0.5 at col 256), so cs[256] = sum_ng/2 and a single reciprocal
  yields the 2/sum_ng scale, fused into the E = exp(.) activation.
- W = 1{iota <= dur-ev} for all 8 tiles is ONE broadcast tensor_tensor
  compare on the otherwise-idle Pool engine, overlapping the input DMAs.
- cum_at = sum(exp * 1{t<=lab}) (== cs[lab] exactly) via per-tile fused
  scalar_tensor_tensor with accumulate, deferred to fill vector-engine gaps.
"""

import os
import numpy as np

import concourse.bacc as bacc
import concourse.mybir as mybir
import concourse.tile as tile
from concourse import bass_utils

N, T = 8192, 256
TPP = T + 2                  # padded row length (sum col + scan-reset col)
N_CORES = 8
NLOC = N // N_CORES          # 1024 rows per core
NT = NLOC // 128             # 8 partition-tiles per core
NCHUNK = 2
QPC = NT // NCHUNK           # 4 tiles per chunk
ALPHA, SIGMA, EPS = 0.5, 0.5, 1e-7

f32 = mybir.dt.float32
f32r = mybir.dt.float32r
Alu = mybir.AluOpType
Act = mybir.ActivationFunctionType

# float32r matmul streams at full PE rate for N>=256; its operand rounding
# contributes ~4e-6 relative error to D (measured offline).
USE_F32R = True
MM_DTYPE = f32r if USE_F32R else f32

_CACHE = {}
LAST_RESULTS = None


def _build():
    nc = bacc.Bacc("TRN2", target_bir_lowering=False, debug=False)

    haz_d = nc.dram_tensor("haz", [NLOC, TPP], f32, kind="ExternalInput")
    # packed per-tile columns: [:, 0:8] = dur - ev, [:, 8:16] = label
    dpk_d = nc.dram_tensor("dpk", [128, 2 * NT], f32, kind="ExternalInput")
    iota_d = nc.dram_tensor("iota", [128, T], f32, kind="ExternalInput")

    D_d = nc.dram_tensor("D", [T, T], f32, kind="ExternalOutput")
    # [:, 0:8] = cumsum(exp(phi)) at label, [:, 8:16] = 2/(rowsum(exp(phi))+1)
    pv_d = nc.dram_tensor("pv", [128, 2 * NT], f32, kind="ExternalOutput")

    CHUNKS = [1, 1, 2, 4]  # graded: first data lands early, rest amortizes

    with tile.TileContext(nc) as tc:
        with (
            tc.tile_pool(name="const", bufs=1) as cpool,
            tc.tile_pool(name="work", bufs=2) as pool,
            tc.tile_pool(name="stage", bufs=1) as spool,
            tc.tile_pool(name="ps", bufs=1, space="PSUM") as pspool,
        ):
            # constants over SWDGE (Pool seq is nearly free); HWDGE carries
            # the hazard chunks
            iota_t = cpool.tile([128, T], f32)
            nc.sync.dma_start(iota_t[:], iota_d[:])
            dpk_t = cpool.tile([128, 2 * NT], f32)
            nc.sync.dma_start(dpk_t[:], dpk_d[:])

            # scan op1 mask: 1.0 body, 0.5 at sum col, 0.0 at reset col
            # (one mask sized for the largest chunk; smaller chunks read a
            # prefix)
            CWMAX = max(CHUNKS) * TPP
            smask_t = cpool.tile([128, CWMAX], f32)
            smask3 = smask_t[:].rearrange("p (q t) -> p q t", q=max(CHUNKS))
            nc.gpsimd.memset(smask_t[:], 1.0)
            nc.gpsimd.memset(smask3[:, :, T : T + 1], 0.5)
            nc.gpsimd.memset(smask3[:, :, T + 1 : TPP], 0.0)

            pv_t = spool.tile([128, 2 * NT], f32)
            D0_ps = pspool.tile([128, T], f32)
            D1_ps = pspool.tile([128, T], f32)

            iota3 = iota_t[:].rearrange("p (one t) -> p one t", one=1)

            # W = 1{iota <= dur - ev} for all 8 tiles in one batched
            # broadcast compare, while the vector engine would otherwise
            # idle waiting for the first hazard chunk (tensor ops are not
            # legal on Pool in hardware)
            W_all = spool.tile([128, NT * T], MM_DTYPE)
            nc.vector.tensor_tensor(
                W_all[:].rearrange("p (q t) -> p q t", q=NT),
                iota3.broadcast_to((128, NT, T)),
                dpk_t[:, 0:NT].broadcast_to((128, NT, T)),
                Alu.is_le,
            )

            haz_v = haz_d[:].rearrange(
                "(c b p) t -> c p b t", c=NT // 2, b=2, p=128
            )

            sttq = []  # deferred low-priority gather work
            for ch in range(NT // 2):
                cw = 2 * TPP
                hazb = pool.tile([128, cw], f32, tag="haz")
                nc.sync.dma_start(
                    hazb[:].rearrange("p (b t) -> p b t", b=2), haz_v[ch]
                )

                # exp(phi) batched per chunk; pad cols give exp(0)=1 (phi
                # max ~5 so no overflow; the reference's gamma shift cancels
                # in every ratio used)
                expb = pool.tile([128, cw], f32, tag="expb", bufs=4)
                nc.scalar.activation(expb[:], hazb[:], Act.Exp)

                for q2 in range(2):
                    q = ch * 2 + q2
                    expa = expb[:, q2 * TPP : (q2 + 1) * TPP]

                    # prefix sum over the padded row (same order as
                    # jnp.cumsum); op1 halves the sum column so one
                    # reciprocal yields 2/sum_ng
                    cs_t = pool.tile([128, TPP], f32, tag="cs")
                    nc.vector.tensor_tensor_scan(
                        cs_t[:], expa, smask_t[:, 0:TPP], 0.0, Alu.add, Alu.mult
                    )

                    # rec2 = 2/sum_ng, straight into pv
                    rec2_c = pv_t[:, NT + q : NT + q + 1]
                    nc.vector.reciprocal(rec2_c, cs_t[:, T : T + 1])

                    # E = exp(cs * 2/sum_ng), scale fused into the activation
                    E_t = pool.tile([128, T], MM_DTYPE, tag="E")
                    nc.scalar.activation(
                        E_t[:], cs_t[:, 0:T], Act.Exp, scale=rec2_c
                    )

                    # D += E^T @ W, t-chunked over PSUM partitions
                    nc.tensor.matmul(
                        D0_ps[:], E_t[:, 0:128], W_all[:, q * T : (q + 1) * T],
                        start=(q == 0), stop=(q == NT - 1),
                    )
                    nc.tensor.matmul(
                        D1_ps[:], E_t[:, 128:T], W_all[:, q * T : (q + 1) * T],
                        start=(q == 0), stop=(q == NT - 1),
                    )
                sttq.append((ch * 2, 2, expb))

            # D halves drain through different engines so the copies and the
            # DMAs overlap (emitted before the gathers for priority; the
            # scheduler interleaves the gathers while the matmuls finish)
            D0_sb = spool.tile([128, T], f32)
            D1_sb = spool.tile([128, T], f32)
            nc.scalar.copy(D0_sb[:], D0_ps[:])
            nc.vector.tensor_copy(D1_sb[:], D1_ps[:])
            nc.sync.dma_start(D_d[0:128, :], D0_sb[:])
            nc.scalar.dma_start(D_d[128:T, :], D1_sb[:])

            # cum_at = cs[lab] == sum(exp * 1{t <= lab}) per tile (fused
            # mask+mult+accumulate). Low priority: fills vector-engine gaps.
            for q0, csize, expb in sttq:
                for q2 in range(csize):
                    q = q0 + q2
                    scr_t = pool.tile([128, T], f32, tag="scr")
                    nc.vector.scalar_tensor_tensor(
                        scr_t[:],
                        iota_t[:],
                        dpk_t[:, NT + q : NT + q + 1],
                        expb[:, q2 * TPP : q2 * TPP + T],
                        Alu.is_le,
                        Alu.mult,
                        accum_out=pv_t[:, q : q + 1],
                    )

            nc.gpsimd.dma_start(pv_d[:], pv_t[:])

    nc.compile()
    return nc


def _get_nc():
    if "nc" not in _CACHE:
        _CACHE["nc"] = _build()
    return _CACHE["nc"]


def _make_in_maps(hazards, duration, event, label):
    iota = np.broadcast_to(
        np.arange(T, dtype=np.float32)[None, :], (128, T)
    ).copy()
    dmef = (duration - event).astype(np.float32)
    labf = label.astype(np.float32)
    hazp = np.zeros((N, TPP), np.float32)
    hazp[:, 0:T] = hazards
    in_maps = []
    for c in range(N_CORES):
        sl = slice(c * NLOC, (c + 1) * NLOC)
        dpk = np.empty((128, 2 * NT), np.float32)
        # column q holds rows [c*NLOC + q*128 : c*NLOC + (q+1)*128)
        dpk[:, 0:NT] = dmef[sl].reshape(NT, 128).T
        dpk[:, NT : 2 * NT] = labf[sl].reshape(NT, 128).T
        in_maps.append(
            {
                "haz": np.ascontiguousarray(hazp[sl]),
                "dpk": dpk,
                "iota": iota,
            }
        )
    return in_maps


def _finish_host(hazards, duration, event, label, D_parts, pv_parts):
    """Host glue: O(n) + O(T^2) arithmetic from the per-core device outputs."""
    n = hazards.shape[0]
    dur = duration.astype(np.int64)
    ev = event.astype(np.int64)
    lab = label.astype(np.int64)

    D = np.zeros((T, T), np.float64)
    cum_at_ng = np.empty(n, np.float32)
    sum_ng = np.empty(n, np.float32)
    for c in range(N_CORES):
        D += D_parts[c].astype(np.float64)
        pv = pv_parts[c]  # [128, 16]
        sl = slice(c * NLOC, (c + 1) * NLOC)
        cum_at_ng[sl] = pv[:, 0:NT].T.reshape(NLOC)
        sum_ng[sl] = np.float32(2.0) / pv[:, NT : 2 * NT].T.reshape(NLOC)

    # rank loss: <D, P> with P the u-weighted (lab, dur) histogram
    cdf_at = cum_at_ng.astype(np.float64) / sum_ng.astype(np.float64)
    u = ev * np.exp(-2.0 * cdf_at)
    P = np.zeros((T, T), np.float64)
    np.add.at(P, (lab, dur), u)
    rank_loss = (D * P).sum() / (float(n) * float(n))

    # nll, following the reference formulas exactly
    gamma = np.maximum(hazards.max(axis=1), 0.0).astype(np.float64)
    eg = np.exp(-gamma)
    sum_ = sum_ng * eg
    cum_at = cum_at_ng * eg
    phi_at = hazards[np.arange(n), lab].astype(np.float64)
    evf = ev.astype(np.float64)
    part1 = (phi_at - gamma) * evf
    part2 = -np.log(np.maximum(sum_, 0.0) + EPS)
    part3 = np.log(np.maximum(sum_ - cum_at, 0.0) + EPS) * (1.0 - evf)
    nll = np.mean(-(part1 + part2 + part3))

    return np.float32(ALPHA * nll + (1.0 - ALPHA) * rank_loss)


def kernel(hazards, duration, event, label):
    global LAST_RESULTS
    hazards = np.asarray(hazards, dtype=np.float32)
    duration = np.asarray(duration)
    event = np.asarray(event)
    label = np.asarray(label)

    nc = _get_nc()
    in_maps = _make_in_maps(hazards, duration, event, label)
    trace = bool(int(os.environ.get("KERNEL_TRACE", "0")))
    res = bass_utils.run_bass_kernel_spmd(
        nc,
        in_maps,
        core_ids=list(range(N_CORES)),
        trace=trace,
        trace_cores=list(range(N_CORES)) if trace else None,
        stitch_traces=False,
    )
    LAST_RESULTS = res
    D_parts = [r["D"] for r in res.results]
    pv_parts = [r["pv"] for r in res.results]
    return _finish_host(hazards, duration, event, label, D_parts, pv_parts)
